# revision 25
# baseline (speedup 1.0000x reference)
"""Trainium2 Bass kernel for MemoryBankNet loss (scatter_memory).

Computes, for inputs/backbone_inputs [256,512], targets [256], memory_features
[100000,512]:
    ce   = cross_entropy(l2norm(inputs) @ mem.T / 0.05, targets)
    dist = (0.007/0.3) * ||l2norm(backbone_inputs) - mem[targets[j//4]]||_F
    out  = ce + dist                                    (f32 scalar)

Distribution: classes (mem rows) are sharded 12288/core across 8 NeuronCores
(tensor parallel over the class axis).  Each core computes its partial softmax
denominator; the tiny [256] partials are combined on host (the "all-reduce" of
the softmax normalizer).  The B-row side terms (target logits for the CE
numerator, distill partials) are exact host numpy over [256,512] gathers --
the host routing of target rows the sharding hint describes.  The ragged
100000-8*12288=1696 remainder classes are summed exactly on host.

Device numerics: memory bank and pre-normalized inputs are quantized host-side
to fp8e4m3 with power-of-2 scales (mem*32, l2norm(inp)*(32/TEMP)), so
PSUM = 1024*logit.

Device pipeline per core (24 substrips of 512 classes; 12 chunks = 6 groups of
4 substrips x 2 b-halves; one 8-bank PSUM tile, chunks ping-pong 4 banks):
  - PE: fp8 DoubleRow matmuls, 2 per bank (256-deep each), ~216ns/MM warm.
    Fill order b0 (k2=0,1), b1,b2,b3 (k2=0), b1,b2,b3 (k2=1): DVE's bank 0
    closes at MM 2 and is drained/freed ~2.2us before its set is reused, so
    the next chunk-pair's refill starts on the early-freed bank and the
    serial ACT -> accumulator-read -> psum-reuse WAR chain stops binding.
  - drain split: DVE handles bank 0 with a Schraudolph bit-trick exp:
    int32(max(psum * 2^13*log2e, 0)) bitcast to f32 equals
    exp(l)*2^-127*rho(frac); tensor_scalar + tensor_reduce, ~1370ns; the
    host rescales those partial columns by kappa = e^(127 ln2 - 104)/rho,
    rho = E[(1+f)2^-f].  ACT exps banks 1-3 (scale 2^-10, bias -104) with
    accum_out (walrus lowers to ACTIVATE + ACTIVATION_READ_ACCUMULATOR,
    ~1540+290ns per chunk).  The last chunk goes three-way (ACT banks 1-2
    only, DVE also drains bank 3 into col 24) so the final
    read -> out-DMA chain is ~1.5us after the last matmul.
  - DMA: per-core HBM delivers only ~240-290 GB/s with all 8 cores pulling
    (chip-level contention).  Every trigger boundary costs ~0.5-1us of arm
    dead-air plus ~0.8us of 16-engine completion-sem straggle, so: the tiny
    stationary wt rides the scalar ring in parallel; the sync ring carries
    six strip-aligned 1MB triggers plus a tiny trailing dummy that makes
    the last strip's sem fire promptly.
  - ~34 junk matmuls on a zeroed tile bridge preamble-end -> first-strip-sem
    (~12-14.5us, core-dependent) so the ~3.4us HAM clock-gate window opens
    during the DMA wait and every real matmul runs at the warm 2.4GHz clock.
"""

import numpy as np
import ml_dtypes

import concourse.bass as bass
import concourse.tile as tile
from concourse import bacc, mybir
from concourse.bass_utils import run_bass_kernel_spmd

F32 = mybir.dt.float32
I32 = mybir.dt.int32
FP8 = mybir.dt.float8e4
BF16 = mybir.dt.bfloat16
AF = mybir.ActivationFunctionType
AX = mybir.AxisListType
ALU = mybir.AluOpType
DR = mybir.MatmulPerfMode.DoubleRow

N_CORES = 8
B, D, C = 256, 512, 100000
CT = 512                     # classes per substrip (one psum bank)
NSUB = 24                    # substrips per core
CS = NSUB * CT               # 12288 device classes per core
SUB_B = 2 * 2 * CT           # 2048 bytes/partition per substrip
TOT_B = NSUB * SUB_B

TEMP = 0.05
SHIFT = 104.0                # fixed log-shift for the ACT exp path
ASCALE = 2.0 ** -10          # undo fp8 scales 32*32 = 1024
DISTILL_SCALE = 0.007 / 0.3
EPS = 1e-12

# Schraudolph path: t = max(v * 2^13*log2e, 0) as int32; bitcast f32 is
# exp(l)*2^-127 * rho(frac).  Host folds 2^-127 -> e^-104 shift and the
# mean ratio rho.
SCHRA_A = 11818.557774962388          # 2^13 * log2(e)
SCHRA_RHO = 1.0406844905028039        # E[(1+f) 2^-f], f ~ U[0,1)
SCHRA_KAPPA = 1.1139462841737636e-07  # e^(127 ln2 - 104) / rho

NCHUNK = 12                  # 6 substrip-groups x 2 b-halves
NWARM = 34                   # junk MMs bridge preamble -> slowest-core
                             # first-strip-sem (~14.5us) for HAM warmth
X_ACT = 1536                 # ACT-path cols per chunk (bank-aligned: DVE's
                             # pass1 then WAR-blocks only bank 3, not bank 2)

_PROGRAM = None
_last_in_maps = None


def _build_program():
    nc = bacc.Bacc("TRN2", target_bir_lowering=False, debug=False,
                   num_devices=N_CORES)
    memT = nc.dram_tensor("memT", [128, TOT_B], FP8, kind="ExternalInput").ap()
    # stationary: [p][h][k2][i][m] fp8, d = k2*256 + i*128 + p, row = h*128+m
    inpT = nc.dram_tensor("inpT", [128, 2, 2, 2, 128], FP8,
                          kind="ExternalInput").ap()
    # per-chunk softmax partials: cols 0..11 ACT path, 12..23 DVE path
    out = nc.dram_tensor("out", [128, 2 * NCHUNK + 1], F32,
                         kind="ExternalOutput").ap()

    with tile.TileContext(nc) as tc:
        _body(tc, nc, memT, inpT, out)

    nc.compile()
    return nc


def _body(tc, nc, memT, inpT, out):
    with (
        tc.tile_pool(name="const", bufs=1) as cpool,
        tc.tile_pool(name="exps", bufs=3) as epool,
        tc.tile_pool(name="t32", bufs=2) as tpool,
        tc.tile_pool(name="psum", bufs=1, space="PSUM") as ppool,
    ):
        wt = cpool.tile([128, 2, 2, 2, 128], FP8, tag="wt", name="wt")
        saccw = cpool.tile([128, 2 * NCHUNK + 1], F32, tag="saccw", name="saccw")
        nbias = cpool.tile([128, 1], F32, tag="nbias", name="nbias")
        jz = cpool.tile([128, 128], F32, tag="jz", name="jz")
        # whole 6.3MB shard is SBUF-resident
        strips = [cpool.tile([128, 4, 2, 2, CT], FP8, tag=f"mt{s}",
                             name=f"mt{s}") for s in range(6)]
        nc.scalar.dma_start(wt[:], inpT)
        for s in range(6):
            nc.sync.dma_start(
                strips[s][:].rearrange("p w k i c -> p (w k i c)"),
                memT[:, s * 4 * SUB_B:(s + 1) * 4 * SUB_B])
        dumt = cpool.tile([128, 32], FP8, tag="dumt", name="dumt")
        nc.sync.dma_start(dumt[:], memT[:, 0:32])
        nc.vector.memset(nbias[:], -SHIFT)
        # junk-MM source on gpsimd (its user code starts earliest; DVE's
        # memset would delay the first junk MM by ~0.5us)
        nc.gpsimd.memset(jz[:], 0.0)

        ps = ppool.tile([128, 8, CT], F32, tag="ps", name="ps")

        # warm-up junk matmuls (see module docstring)
        jw = jz[:].bitcast(FP8).rearrange("p (k m) -> p k m", k=2)[:, :, 0:128]
        jr = jz[:].bitcast(FP8).rearrange("p (k c) -> p k c", k=2)
        for _ in range(NWARM):
            nc.tensor.matmul(ps[:, 0, 0:256], jw, jr, start=True, stop=True,
                             perf_mode=DR, skip_group_check=True)

        # (bank, k2) fill order: DVE's bank 0 closes at MM 2, so the next
        # chunk-pair's refill starts on an early-freed bank and the
        # ACT -> accumulator-read -> psum-reuse WAR chain stops binding
        FILL = [(0, 0), (0, 1), (1, 0), (2, 0), (3, 0), (1, 1), (2, 1), (3, 1)]
        for c in range(NCHUNK):
            g, h = c // 2, c % 2
            b0 = 4 * (c % 2)         # chunk's first psum bank (ping-pong)
            for jj, k2 in FILL:
                su = 4 * g + jj
                nc.tensor.matmul(
                    ps[:, b0 + jj, :],
                    wt[:, h, k2],
                    strips[su // 4][:, su % 4, k2],
                    start=(k2 == 0), stop=(k2 == 1),
                    perf_mode=DR, skip_group_check=True)

            flat = ps[:, b0:b0 + 4, :].rearrange("p b c -> p (b c)")

            tb = tpool.tile([128, CT], I32, tag="tb", name="tb")
            nc.vector.tensor_scalar(
                tb[:], flat[:, 0:CT], SCHRA_A, 0.0, ALU.mult, ALU.max)
            nc.vector.reduce_sum(
                saccw[:, NCHUNK + c:NCHUNK + c + 1],
                tb[:].bitcast(F32), axis=AX.X)

            # last chunk: ACT takes only banks 1-2 (close at MM 7) and DVE
            # also drains bank 3 into the extra col 24, so the final
            # read -> out chain is ~1.5us instead of ~1.9us after MM end
            last = c == NCHUNK - 1
            xhi = 3 * CT if last else 2048
            ex = epool.tile([128, 3 * CT], BF16, tag="ex", name="ex")
            nc.scalar.activation(
                ex[:, 0:xhi - CT], flat[:, CT:xhi],
                AF.Exp, bias=nbias[:], scale=ASCALE,
                accum_out=saccw[:, c:c + 1])
            if last:
                tb2 = tpool.tile([128, CT], I32, tag="tb", name="tb2")
                nc.vector.tensor_scalar(
                    tb2[:], flat[:, 3 * CT:2048], SCHRA_A, 0.0,
                    ALU.mult, ALU.max)
                nc.vector.reduce_sum(
                    saccw[:, 2 * NCHUNK:2 * NCHUNK + 1],
                    tb2[:].bitcast(F32), axis=AX.X)

        # out rides the ACT ring: the ACT engine is idle and local when the
        # last accumulator read lands, skipping the cross-engine sem hop
        nc.scalar.dma_start(out, saccw[:])


def _get_program():
    global _PROGRAM
    if _PROGRAM is None:
        _PROGRAM = _build_program()
    return _PROGRAM


def _quant_fp8(x):
    return np.clip(x, -240.0, 240.0).astype(ml_dtypes.float8_e4m3)


def kernel(backbone_inputs, inputs, targets, memory_features, **_unused):
    x = np.ascontiguousarray(inputs, dtype=np.float32)
    bb = np.ascontiguousarray(backbone_inputs, dtype=np.float32)
    mem = np.ascontiguousarray(memory_features, dtype=np.float32)
    tgt = np.asarray(targets).astype(np.int64)

    xn = x / np.maximum(np.linalg.norm(x, axis=1, keepdims=True), EPS)

    # stationary: value[p,h,k2,i,m] = (xn*640)[h*128+m, k2*256+i*128+p]
    qi = _quant_fp8(xn * (32.0 / TEMP))
    spt8 = np.ascontiguousarray(
        qi.reshape(2, 128, 2, 2, 128).transpose(4, 0, 2, 3, 1))

    qm = _quant_fp8(mem * 32.0)

    nc = _get_program()
    in_maps = []
    for c in range(N_CORES):
        sh = qm[c * CS:(c + 1) * CS].reshape(NSUB, CT, 2, 2, 128)
        shard = np.ascontiguousarray(sh.transpose(4, 0, 2, 3, 1)).reshape(128, -1)
        in_maps.append({"memT": shard, "inpT": spt8})
    global _last_in_maps
    _last_in_maps = in_maps
    results = run_bass_kernel_spmd(nc, in_maps, core_ids=list(range(N_CORES)))

    s_tot = np.zeros(B, dtype=np.float64)
    for r in results.results:
        o = r["out"].astype(np.float64)      # [128, 24] per-chunk partials
        acol = o[:, 0:NCHUNK]                # ACT path, shift -104
        dcol = o[:, NCHUNK:2 * NCHUNK] * SCHRA_KAPPA   # DVE Schraudolph path
        tot = acol + dcol                    # [128, chunk c -> h = c % 2]
        s_tot[0:128] += tot[:, 0::2].sum(axis=1)
        s_tot[128:256] += tot[:, 1::2].sum(axis=1) \
            + o[:, 2 * NCHUNK] * SCHRA_KAPPA   # c11's bank-3 DVE partial
    # remainder classes (C - 8*CS = 1696): exact on host
    lt = (xn @ mem[N_CORES * CS:].T.astype(np.float64)) / TEMP
    s_tot += np.exp(lt - SHIFT).sum(axis=1)

    # host: exact B-row side terms (target-row routing per sharding hint)
    lse = SHIFT + np.log(s_tot)
    tl = np.einsum("bd,bd->b", xn, mem[tgt], dtype=np.float64) / TEMP
    ce = float(np.mean(lse - tl))
    bbn = bb / np.maximum(np.linalg.norm(bb, axis=1, keepdims=True), EPS)
    g2 = mem[tgt[np.arange(B) // 4]]
    dist = DISTILL_SCALE * float(
        np.sqrt(((bbn.astype(np.float64) - g2) ** 2).sum()))
    return np.asarray(ce + dist, dtype=np.float32)
